# revision 23
# baseline (speedup 1.0000x reference)
"""Contrastive-loss kernel for Trainium2 (8 NeuronCores, SPMD data-parallel).

Math (from the reference):
    diag_A_is = (A_is_t + A_is_t_14 + A_is_t_28)[i, i, :]        # [B, D]
    diag_A_em = (A_em_t + A_em_t_14 + A_em_t_28)[i, i, :]        # [B, D]
    loss = sum_b relu( sum_d (0.4*m + 0.6*tr_m) * (diag_A_is - diag_A_em) )

Only the diagonals A[i, i, :] of the six [B, B, D] tensors are touched
(1/256th of the data).  Sharding strategy: batch-dim data parallel across
the 8 cores — the host gathers the diagonal rows (pure data movement) and
ships each core its 32 rows of the eight [B, D] operands packed into one
1.06 MB fp32 buffer; all arithmetic runs on-device.  Per-core partial
losses are summed on the host (8 scalars).

Device-side layout per core (SBUF tile xt [128 partitions x 2080 f32]):
  each [32, 1024] operand block is flattened row-major to [128, 256]
  (partition p = 4*row + quarter, 256 contiguous d's per partition).
  cols:  m 0:256 | tr 256:512 | E 512:544 | is0 544:800 | em0 800:1056 |
         is1 1056:1312 | em1 1312:1568 | is2a 1568:1696 | em2a 1696:1824 |
         is2b 1824:1952 | em2b 1952:2080
  E[p, b] = 1.0 iff p // 4 == b — used as the matmul rhs to sum the four
  per-partition quarter-row dots of each batch row (partition reduction).

Factoring: 0.4*m + 0.6*tr_m = 0.4 * (m + 1.5*tr_m) and
relu(0.4 x) = 0.4 relu(x), so the 0.4 is applied host-side to the scalar.

The DRAM input "x" is chunk-major (five contiguous [128, cols] blocks) so
every DMA reads one sequential DRAM range.  5 load DMAs spread over both
HWDGE rings (sync + scalar); DVE work is pipelined per chunk; each chunk's
per-partition dot lands in rowq_parts and is folded over partitions by
PSUM-accumulating 1-column matmuls against E; a final fused
relu+accumulate produces the scalar.

Raw bass (no TileContext) on purpose: this walrus build enforces a tiny
per-instruction sync-wait limit (Tile's kernel-tail Drain needs one wait
per live semaphore and fails codegen at 4), and Tile's epilogue barrier
costs several microseconds.  With explicit blocks every wait is its own
instruction.  Custom-DVE ops (tensor_tensor_reduce etc.) are avoided —
they lower to InstISA, which this walrus rejects ("ISA wrong length").
"""

import numpy as np

import concourse.bass as bass
import concourse.mybir as mybir
from concourse.bass_utils import run_bass_kernel_spmd

B = 256
D = 1024
N_CORES = 8
ROWS_PER_CORE = B // N_CORES  # 32
BLK = 256  # free-dim width of one packed [32, 1024] operand block
E_COLS = ROWS_PER_CORE  # 32
FREE = 8 * BLK + E_COLS  # 2080
O = 2 * BLK + E_COLS  # 544 = cols in chunk 0 (m, tr, E)
# chunk-major DRAM layout: chunk i is a contiguous [128, CHUNK_COLS[i]] block
CHUNK_COLS = [O, 2 * BLK, 2 * BLK, BLK, BLK]
CHUNK_OFF = [0]
for _c in CHUNK_COLS:
    CHUNK_OFF.append(CHUNK_OFF[-1] + 128 * _c)

_NC_CACHE = None


def build_nc() -> bass.Bass:
    f32 = mybir.dt.float32
    Alu = mybir.AluOpType

    nc = bass.Bass()
    x = nc.dram_tensor("x", [128 * FREE], f32, kind="ExternalInput")
    out_d = nc.dram_tensor("out", [1, 1], f32, kind="ExternalOutput")

    def x_chunk(i):
        return x[CHUNK_OFF[i] : CHUNK_OFF[i + 1]].rearrange(
            "(p f) -> p f", f=CHUNK_COLS[i]
        )

    with (
        nc.sbuf_tensor("xt", [128, FREE], f32) as xt,
        nc.sbuf_tensor("w", [128, BLK], f32) as w,
        nc.sbuf_tensor("diff", [128, 3 * BLK], f32) as diff,
        nc.sbuf_tensor("prod", [128, 3 * BLK], f32) as prod,
        nc.sbuf_tensor("rowq_parts", [128, 4], f32) as rowq_parts,
        nc.sbuf_tensor("srelu", [1, E_COLS], f32) as srelu,
        nc.sbuf_tensor("total", [1, 1], f32) as total,
        nc.psum_tensor("ps", [1, E_COLS], f32) as ps,
        nc.semaphore("s1") as s1,  # sync ring: chunk0 load (+out store)
        nc.semaphore("s2") as s2,  # sync ring: chunk2 (is1 em1)
        nc.semaphore("a1") as a1,  # scalar ring: chunk1 (is0 em0)
        nc.semaphore("a2") as a2,  # scalar ring: chunk3 (is2a em2a)
        nc.semaphore("a3") as a3,  # scalar ring: chunk4 (is2b em2b)
        nc.semaphore("v_sem") as v_sem,
        nc.semaphore("pe_sem") as pe_sem,
        nc.Block() as block,
    ):
        m_ap = xt[:, 0:BLK]
        tr_ap = xt[:, BLK : 2 * BLK]
        e_ap = xt[:, 2 * BLK : O]
        # (is, em, w-slice, diff/prod col, width) per compute step
        steps = [
            (xt[:, 544:800], xt[:, 800:1056], w[:, :], 0, BLK),
            (xt[:, 1056:1312], xt[:, 1312:1568], w[:, :], BLK, BLK),
            (xt[:, 1568:1696], xt[:, 1696:1824], w[:, 0:128], 2 * BLK, 128),
            (xt[:, 1824:1952], xt[:, 1952:2080], w[:, 128:256], 2 * BLK + 128, 128),
        ]
        dma_waits = [(a1, 16), (s2, 16), (a2, 16), (a3, 16)]

        @block.sync
        def _(sync):
            sync.dma_start(out=xt[:, 0:O], in_=x_chunk(0)).then_inc(s1, 16)
            sync.dma_start(out=xt[:, 1056:1568], in_=x_chunk(2)).then_inc(s2, 16)
            sync.wait_ge(v_sem, 10)
            sync.dma_start(out=out_d[:], in_=total[:]).then_inc(s1, 16)
            sync.wait_ge(s1, 32)

        @block.scalar
        def _(scalar):
            scalar.dma_start(out=xt[:, 544:1056], in_=x_chunk(1)).then_inc(a1, 16)
            scalar.dma_start(out=xt[:, 1568:1824], in_=x_chunk(3)).then_inc(a2, 16)
            scalar.dma_start(out=xt[:, 1824:2080], in_=x_chunk(4)).then_inc(a3, 16)

        @block.vector
        def _(vector):
            # w = m + 1.5 * tr_m
            vector.wait_ge(s1, 16)
            nc.vector.scalar_tensor_tensor(
                out=w[:], in0=tr_ap, scalar=1.5, in1=m_ap,
                op0=Alu.mult, op1=Alu.add,
            ).then_inc(v_sem, 1)
            # per chunk: diff = is - em, then fused prod = diff * w with
            # accum_out = per-partition sum -> rowq_parts[:, i]
            for i, ((is_i, em_i, w_i, col, wd), dw) in enumerate(
                zip(steps, dma_waits)
            ):
                vector.wait_ge(*dw)
                nc.vector.tensor_sub(
                    diff[:, col : col + wd], is_i, em_i
                ).then_inc(v_sem, 1)
                vector.wait_ge(v_sem, 2 * i + 2)
                nc.vector.scalar_tensor_tensor(
                    out=prod[:, col : col + wd], in0=diff[:, col : col + wd],
                    scalar=1.0, in1=w_i, op0=Alu.mult, op1=Alu.mult,
                    accum_out=rowq_parts[:, i : i + 1],
                ).then_inc(v_sem, 1)
            # relu the 32 per-row sums (in PSUM), accumulate to one scalar
            vector.wait_ge(pe_sem, 1)
            nc.vector.tensor_scalar(
                out=srelu[:], in0=ps[:], scalar1=0.0, scalar2=None,
                op0=Alu.max, op1=Alu.add, accum_out=total[:],
            ).then_inc(v_sem, 1)

        @block.tensor
        def _(tensor):
            tensor.wait_ge(s1, 16)
            # ps[1, 32] += rowq_parts[:, i]^T @ E — PSUM-accumulate the four
            # chunk dots while folding each row's 4 partition-quarters
            for i in range(4):
                tensor.wait_ge(v_sem, 2 * i + 3)
                mm = nc.tensor.matmul(
                    ps[:], rowq_parts[:, i : i + 1], e_ap,
                    start=(i == 0), stop=(i == 3),
                )
                if i == 3:
                    mm.then_inc(pe_sem, 1)

    return nc


def pack_inputs(A_is_t, A_is_t_14, A_is_t_28, A_em_t, A_em_t_14, A_em_t_28, m, tr_m):
    idx = np.arange(B)

    def diag(a):
        return np.asarray(a)[idx, idx]  # [B, D] gather of the used diagonal

    def blk(a):  # per-core [128, 256] flattening of a [B, D] operand
        return np.asarray(a, dtype=np.float32).reshape(N_CORES, 128, BLK)

    is2 = blk(diag(A_is_t_28))
    em2 = blk(diag(A_em_t_28))
    X = np.empty((N_CORES, 128, FREE), dtype=np.float32)
    X[:, :, 0:BLK] = blk(m)
    X[:, :, BLK : 2 * BLK] = blk(tr_m)
    X[:, :, 2 * BLK : O] = np.repeat(np.eye(E_COLS, dtype=np.float32), 4, axis=0)
    X[:, :, 544:800] = blk(diag(A_is_t))
    X[:, :, 800:1056] = blk(diag(A_em_t))
    X[:, :, 1056:1312] = blk(diag(A_is_t_14))
    X[:, :, 1312:1568] = blk(diag(A_em_t_14))
    X[:, :, 1568:1696] = is2[:, :, :128]
    X[:, :, 1696:1824] = em2[:, :, :128]
    X[:, :, 1824:1952] = is2[:, :, 128:]
    X[:, :, 1952:2080] = em2[:, :, 128:]
    # chunk-major flat layout: each DMA reads one contiguous DRAM range
    bounds = [0, O, 1056, 1568, 1824, FREE]
    return [
        {
            "x": np.concatenate(
                [X[c, :, bounds[i] : bounds[i + 1]].ravel() for i in range(5)]
            )
        }
        for c in range(N_CORES)
    ]


def run(in_maps, **kwargs):
    global _NC_CACHE
    if _NC_CACHE is None:
        _NC_CACHE = build_nc()
    return run_bass_kernel_spmd(
        _NC_CACHE, in_maps, core_ids=list(range(N_CORES)), **kwargs
    )


def kernel(**inputs) -> np.ndarray:
    res = run(pack_inputs(**inputs))
    total = 0.4 * sum(float(r["out"][0, 0]) for r in res.results)
    return np.array([total], dtype=np.float32)
